# revision 19
# baseline (speedup 1.0000x reference)
"""Trainium2 Bass kernel for nn_BankedDenoiser.

Data-parallel over batch: 8 batch elements -> 8 NeuronCores, one element per
core, no collectives. Activations are kept feature-major (hT [D, S]) in SBUF so
every y = x @ W becomes matmul(lhsT=W_chunk, rhs=hT) with weights in their
natural DRAM layout. Matmuls run in bf16 (fp8e4m3 DoubleRow where flagged)
with fp32 PSUM accumulation. All weights are converted to bf16/fp8 on the
host and DMA'd directly (no on-chip staging), with scalar factors
(1/sqrt(DH), BETA*ETA/sqrt(DH), 1/sqrt(D)) folded into the host weights, and
the positional encoding + t_embed + b_in folded into one host tensor.

Attention: scores are produced transposed (scoresT [key, query]) so the
exponentiated weights feed the A@V matmul as rhs; the softmax denominator
comes from augmenting token-major V with a ones column, and per-query
normalization is applied via a rank-1 ones-broadcast matmul of the
reciprocal row. exp() runs without max-subtraction (scores are bounded for
this module's weight scale) straight out of PSUM on ScalarE. A@V uses
fp8e4m3 DoubleRow over token-chunk pairs (exp weights quantize harmlessly).

LayerNorm (feature-major): sum / sum-of-squares via ones-column matmuls on
the PE, rstd = exp(-0.5 * ln(var/D + eps)); mean/rstd rows broadcast with
rank-1 matmuls. Squares and LN-apply multiplies run on Pool (gpsimd) to keep
ScalarE free for exp.

Bank-attention key biases (gamma*ln(Size)-sig/2-c*|phi|^2) are tiny [M]
vectors computed on the host.

Router top-4: Max8 gives each token's top-8 logits sorted; the dense
dispatch matrix P^T[m, s] = exp(logit - logsumexp_top4) * (logit >= midgap)
is built transposed and applied as a matmul against token-major Z.
"""

import numpy as np
import ml_dtypes

B, S, IN_DIM, D, H, L, M, TOPK = 8, 1024, 256, 512, 8, 4, 256, 4
DFF = 2048
DH = D // H
TAU, GAMMA, BETA, ETA = 1.0, 0.3, 1.0, 1.0
P = 128
KD = D // P          # 4 feature chunks of 128
SC = S // 512        # 2 column chunks of 512
SCH = S // P         # 8 token chunks of 128
NF = DFF // P        # 16 dff chunks
C2 = BETA / (TAU * D)

# fp8e4m3 DoubleRow toggles per site
FP8_QK = True        # q,k projections from fp8 h
FP8_QK_DR = False    # use DoubleRow for the q,k matmuls
FP8_AV = False        # attention A@V with fp8 exp-weights and V
FP8_FFN1 = False      # ffn W1 matmul from fp8 h
FP8_FFN2 = False      # ffn W2 matmul from fp8 relu output

_CACHE = {}


def _build(flags, reps=1):
    import concourse.tile as tile
    from concourse import bacc, mybir
    from concourse.masks import make_identity

    f32 = mybir.dt.float32
    bf16 = mybir.dt.bfloat16
    f8 = mybir.dt.float8e4
    AF = mybir.ActivationFunctionType
    OP = mybir.AluOpType
    DR = mybir.MatmulPerfMode.DoubleRow

    assert not flags["mask"], "non-trivial mask not supported"
    assert not flags["bqkv"], "nonzero enc qkv bias not supported"

    nc = bacc.Bacc("TRN2", target_bir_lowering=False, debug=False, num_devices=8)

    def din(name, shape, dt=bf16):
        return nc.dram_tensor(name, shape, dt, kind="ExternalInput").ap()

    xT_d = din("xT", [IN_DIM, S])
    pep_d = din("pep", [D, S])          # peT + t_embed[b] + b_in (host-folded)
    phiT_d = din("phiT", [D, M])
    phi2c_d = din("phi2c", [D, M])      # 2*C2*phiT
    drow_d = din("drow", [1, M], f32)   # key bias row (host-computed)
    p2m_d = din("p2m", [1, M], f32)     # -C2*|phi|^2 row (host-computed)
    win_d = din("win", [IN_DIM, D])
    if FP8_QK:
        wqk_d = din("wqk8", [L, D, 2 * D], f8)   # q cols pre-scaled 1/sqrt(DH)
        wv_d = din("wvb", [L, D, D])
    else:
        wqkv_d = din("wqkv", [L, D, 3 * D])
    wo_d = din("wo4", [L, D, D])
    w1_d = din("w18", [L, D, DFF], f8 if FP8_FFN1 else bf16)
    w2_d = din("w28", [L, DFF, D], f8 if FP8_FFN2 else bf16)
    saq_d = din("saq", [D, D])
    sak_d = din("sak", [D, D])          # pre-scaled BETA*ETA/sqrt(DH)
    sav_d = din("sav", [D, D])
    sao_d = din("sao", [D, D])
    rtq_d = din("rtq", [D, D])          # pre-scaled 1/sqrt(D)
    wout_d = din("wout", [D, IN_DIM])
    bo_d = din("bo4", [L, D], f32) if flags["bo"] else None
    b1_d = din("b14", [L, DFF], f32) if flags["b1"] else None
    b2_d = din("b24", [L, D], f32) if flags["b2"] else None
    ln_d = din("lnp", [L, 4, D], f32) if flags["ln"] else None
    bout_d = din("bout", [IN_DIM], f32) if flags["bout"] else None
    out_d = nc.dram_tensor("outT", [IN_DIM, S], f32, kind="ExternalOutput").ap()

    with tile.TileContext(nc) as tc:
        with (tc.tile_pool(name="const", bufs=1) as cpool,
              tc.tile_pool(name="keep", bufs=1) as keep,
              tc.tile_pool(name="pswide", bufs=2, space="PSUM") as ps_wide,
              tc.tile_pool(name="psmm", bufs=1, space="PSUM") as ps_mm,
              tc.tile_pool(name="pssm", bufs=1, space="PSUM") as ps_sm,
              tc.tile_pool(name="dram", bufs=2, space="DRAM") as dpool):

            ident = cpool.tile([P, P], f32)
            make_identity(nc, ident[:])
            ones_r = cpool.tile([1, D], f32)
            nc.vector.memset(ones_r[:], 1.0)
            ones_cb = cpool.tile([P, 1], bf16)
            nc.vector.memset(ones_cb[:], 1.0)
            eps5_r = cpool.tile([1, 1], f32)
            nc.vector.memset(eps5_r[:], 1e-5)

            _n = [0]

            def pwide():
                _n[0] += 1
                return ps_wide.tile([P, 1024], f32, tag="wide", bufs=2,
                                    name=f"wps{_n[0]}")

            def pmm():
                _n[0] += 1
                return ps_mm.tile([P, 512], f32, tag="mm", bufs=1,
                                  name=f"mmps{_n[0]}")

            def psmall():
                _n[0] += 1
                return ps_sm.tile([P, 512], f32, tag="small", bufs=3,
                                  name=f"sps{_n[0]}")

            # DRAM [K, N] -> SBUF [P, K//P, N], split into nsplit DMAs
            def wload(pool, dram2d, K, N, dt, tag, bufs=1, nsplit=4):
                ko_n = K // P
                w = pool.tile([P, ko_n, N], dt, tag=tag, bufs=bufs)
                src = dram2d.rearrange("(ko p) m -> p ko m", p=P)
                step = max(1, ko_n // nsplit)
                for c0 in range(0, ko_n, step):
                    cn = min(step, ko_n - c0)
                    nc.sync.dma_start(w[:, c0:c0 + cn, :], src[:, c0:c0 + cn, :])
                return w

            def col_from(dram1d, n, tag):
                t = cpool.tile([P, n // P], f32, tag=tag)
                nc.sync.dma_start(t[:], dram1d.rearrange("(o p) -> p o", p=P))
                return t

            if flags["bo"]:
                bo_c = [col_from(bo_d[l], D, f"bo{l}") for l in range(L)]
            if flags["b1"]:
                b1_c = [col_from(b1_d[l], DFF, f"b1{l}") for l in range(L)]
            if flags["b2"]:
                b2_c = [col_from(b2_d[l], D, f"b2{l}") for l in range(L)]
            if flags["ln"]:
                ln_c = [[col_from(ln_d[l, j], D, f"ln{l}_{j}") for j in range(4)]
                        for l in range(L)]
            if flags["bout"]:
                bout_c = col_from(bout_d, IN_DIM, "boutc")

            # persistent across phases
            phiT_b = keep.tile([P, KD, M], bf16, tag="phiTb")
            nc.sync.dma_start(phiT_b[:], phiT_d.rearrange("(ko p) m -> p ko m", p=P))
            z_sb = keep.tile([P, 2, D], bf16, tag="ztok")

            def inproj_phase(ip):
                win_w = wload(ip, win_d, IN_DIM, D, bf16, "win", nsplit=2)
                xT_b = ip.tile([P, 2, S], bf16, tag="xTb")
                nc.sync.dma_start(xT_b[:], xT_d.rearrange("(ko p) s -> p ko s", p=P))
                pep_sb = ip.tile([P, KD, S], bf16, tag="pep")
                nc.sync.dma_start(pep_sb[:, 0:2, :],
                                  pep_d.rearrange("(ko p) s -> p ko s", p=P)[:, 0:2, :])
                nc.sync.dma_start(pep_sb[:, 2:4, :],
                                  pep_d.rearrange("(ko p) s -> p ko s", p=P)[:, 2:4, :])
                h_sb = keep.tile([P, KD, S], bf16, tag="hT", bufs=2)
                for m in range(KD):
                    ps = pwide()
                    for k in range(2):
                        for sc in range(SC):
                            sl = slice(sc * 512, (sc + 1) * 512)
                            nc.tensor.matmul(ps[:, sl], win_w[:, k, m * P:(m + 1) * P],
                                             xT_b[:, k, sl],
                                             start=(k == 0), stop=(k == 1))
                    nc.vector.tensor_add(h_sb[:, m, :], ps[:], pep_sb[:, m, :])
                return h_sb

            def bank_phase(bp):
                saq_w = wload(bp, saq_d, D, D, bf16, "saq", nsplit=2)
                sak_w = wload(bp, sak_d, D, D, bf16, "sak", nsplit=2)
                sav_w = wload(bp, sav_d, D, D, bf16, "sav", nsplit=2)
                sao_w = wload(bp, sao_d, D, D, bf16, "sao", nsplit=2)
                phi2c_b = bp.tile([P, KD, M], bf16, tag="phi2c")
                nc.sync.dma_start(phi2c_b[:],
                                  phi2c_d.rearrange("(ko p) m -> p ko m", p=P))
                d_row = bp.tile([1, M], f32, tag="drow")
                nc.sync.dma_start(d_row[:], drow_d)
                p2m_row = bp.tile([1, M], f32, tag="p2mrow")
                nc.sync.dma_start(p2m_row[:], p2m_d)

                qTb = bp.tile([P, KD, M], bf16, tag="qTb")
                kTb = bp.tile([P, KD, M], bf16, tag="kTb")
                for dst, wmat in ((qTb, saq_w), (kTb, sak_w)):
                    for m in range(KD):
                        ps = pmm()
                        for k in range(KD):
                            nc.tensor.matmul(ps[:, :M],
                                             wmat[:, k, m * P:(m + 1) * P],
                                             phiT_b[:, k, :],
                                             start=(k == 0), stop=(k == KD - 1))
                        nc.vector.tensor_copy(dst[:, m, :], ps[:, :M])
                vb_aug = bp.tile([P, 2, H, DH + 1], bf16, tag="vbaug")
                nc.vector.memset(vb_aug[:], 1.0)
                for nch in range(2):
                    ps = pmm()
                    for k in range(KD):
                        nc.tensor.matmul(ps[:], phiT_b[:, k, nch * P:(nch + 1) * P],
                                         sav_w[:, k, :],
                                         start=(k == 0), stop=(k == KD - 1))
                    nc.vector.tensor_copy(
                        vb_aug[:, nch, :, 0:DH],
                        ps[:].rearrange("p (h c) -> p h c", c=DH))

                oTb = bp.tile([P, KD, M], bf16, tag="oTb")
                for h in range(H):
                    p0, ko = DH * (h % 2), h // 2
                    eb = bp.tile([P, 2, M], bf16, tag="expb", bufs=2)
                    for nch in range(2):
                        ps = pmm()
                        for k in range(KD):
                            nc.tensor.matmul(ps[:, :M],
                                             phiT_b[:, k, nch * P:(nch + 1) * P],
                                             phi2c_b[:, k, :],
                                             start=(k == 0), stop=False)
                        nc.tensor.matmul(ps[:, :M], d_row[:, nch * P:(nch + 1) * P],
                                         ones_r[:, :M], start=False, stop=False)
                        nc.tensor.matmul(ps[:, :M], ones_r[:, :P], p2m_row[:],
                                         start=False, stop=False)
                        nc.tensor.matmul(ps[:, :M],
                                         kTb[p0:p0 + DH, ko, nch * P:(nch + 1) * P],
                                         qTb[p0:p0 + DH, ko, :],
                                         start=False, stop=True)
                        nc.scalar.activation(eb[:, nch, :], ps[:, :M], AF.Exp)
                    zb = psmall()
                    for nch in range(2):
                        nc.tensor.matmul(zb[:DH + 1, :M], vb_aug[:, nch, h, :],
                                         eb[:, nch, :],
                                         start=(nch == 0), stop=(nch == 1))
                    rb = bp.tile([1, M], f32, tag="recb", bufs=2)
                    nc.vector.reciprocal(rb[:], zb[DH:DH + 1, :M])
                    bc = psmall()
                    nc.tensor.matmul(bc[:DH, :M], ones_r[:, :DH], rb[:],
                                     start=True, stop=True)
                    bcs = bp.tile([DH, M], bf16, tag="bcsb", bufs=2)
                    nc.scalar.copy(bcs[:], bc[:DH, :M])
                    nc.vector.tensor_mul(oTb[p0:p0 + DH, ko, :], zb[0:DH, :M], bcs[:])
                for mch in range(2):
                    ps = pmm()
                    for k in range(KD):
                        nc.tensor.matmul(ps[:], oTb[:, k, mch * P:(mch + 1) * P],
                                         sao_w[:, k, :],
                                         start=(k == 0), stop=(k == KD - 1))
                    nc.vector.tensor_copy(z_sb[:, mch, :], ps[:])

            def emit_ln(ep, r_t, rsq_t, lidx, lnoff):
                # r_t: bf16 [P, KD, S]; rsq_t: bf16 [P, KD, S] (squares)
                out = keep.tile([P, KD, S], bf16, tag="hT", bufs=2)
                for sc in range(SC):
                    sl = slice(sc * 512, (sc + 1) * 512)
                    ps1 = psmall()
                    for k in range(KD):
                        nc.tensor.matmul(ps1[:1, :], ones_cb[:], r_t[:, k, sl],
                                         start=(k == 0), stop=(k == KD - 1))
                    ps2 = psmall()
                    for k in range(KD):
                        nc.tensor.matmul(ps2[:1, :], ones_cb[:], rsq_t[:, k, sl],
                                         start=(k == 0), stop=(k == KD - 1))
                    mu_row = ep.tile([1, 512], f32, tag="murow", bufs=2)
                    nc.vector.tensor_scalar_mul(mu_row[:], ps1[:1, :], 1.0 / D)
                    tr = ep.tile([1, 512], f32, tag="tmprow", bufs=2)
                    nc.vector.tensor_mul(tr[:], ps1[:1, :], mu_row[:])
                    var_row = ep.tile([1, 512], f32, tag="varrow", bufs=2)
                    nc.vector.tensor_sub(var_row[:], ps2[:1, :], tr[:])
                    nc.scalar.activation(var_row[:], var_row[:], AF.Ln,
                                         bias=eps5_r[:], scale=1.0 / D)
                    rstd_row = ep.tile([1, 512], f32, tag="rstdrow", bufs=2)
                    nc.scalar.activation(rstd_row[:], var_row[:], AF.Exp, scale=-0.5)
                    mb_ps = psmall()
                    nc.tensor.matmul(mb_ps[:], ones_r[:, :P], mu_row[:],
                                     start=True, stop=True)
                    mb = ep.tile([P, 512], bf16, tag="mubc", bufs=2)
                    nc.vector.tensor_copy(mb[:], mb_ps[:])
                    rb_ps = psmall()
                    nc.tensor.matmul(rb_ps[:], ones_r[:, :P], rstd_row[:],
                                     start=True, stop=True)
                    rbt = ep.tile([P, 512], bf16, tag="rstdbc", bufs=2)
                    nc.vector.tensor_copy(rbt[:], rb_ps[:])
                    for k in range(KD):
                        t1 = ep.tile([P, 512], bf16, tag="lnt1", bufs=4)
                        nc.vector.tensor_sub(t1[:], r_t[:, k, sl], mb[:])
                        if flags["ln"]:
                            t2 = ep.tile([P, 512], bf16, tag="lnt2", bufs=2)
                            nc.gpsimd.tensor_mul(t2[:], t1[:], rbt[:])
                            nc.vector.tensor_scalar(
                                out[:, k, sl], t2[:],
                                ln_c[lidx][lnoff][:, k:k + 1],
                                ln_c[lidx][lnoff + 1][:, k:k + 1], OP.mult, OP.add)
                        else:
                            nc.gpsimd.tensor_mul(out[:, k, sl], t1[:], rbt[:])
                return out

            def encoder_layer(ep, l, h_sb):
                if FP8_QK:
                    wqk_w = wload(ep, wqk_d[l], D, 2 * D, f8, "wqk8", bufs=2,
                                  nsplit=2)
                    wv_w = wload(ep, wv_d[l], D, D, bf16, "wvb", bufs=1, nsplit=2)
                else:
                    wqkv_w = wload(ep, wqkv_d[l], D, 3 * D, bf16, "wqkv", bufs=1,
                                   nsplit=4)
                wo_w = wload(ep, wo_d[l], D, D, bf16, "wo", bufs=1, nsplit=2)
                w1_w = wload(ep, w1_d[l], D, DFF, f8 if FP8_FFN1 else bf16,
                             "w1", bufs=2 if FP8_FFN1 else 1, nsplit=4)
                w2_w = wload(ep, w2_d[l], DFF, D, f8 if FP8_FFN2 else bf16,
                             "w2", bufs=2 if FP8_FFN2 else 1, nsplit=4)
                if FP8_QK or FP8_FFN1:
                    h8 = ep.tile([P, KD, S], f8, tag="h8", bufs=1)
                    for m in range(KD):
                        nc.gpsimd.tensor_copy(h8[:, m, :], h_sb[:, m, :])
                qT = ep.tile([P, KD, S], bf16, tag="qT")
                kT = ep.tile([P, KD, S], bf16, tag="kT")
                for which, dst in ((0, qT), (1, kT)):
                    off = which * D
                    for m in range(KD):
                        ps = pwide()
                        for sc in range(SC):
                            sl = slice(sc * 512, (sc + 1) * 512)
                            if FP8_QK and FP8_QK_DR:
                                for k in range(0, KD, 2):
                                    nc.tensor.matmul(
                                        ps[:, sl],
                                        wqk_w[:, k:k + 2, off + m * P:off + (m + 1) * P],
                                        h8[:, k:k + 2, sl], perf_mode=DR,
                                        start=(k == 0), stop=(k == KD - 2))
                            elif FP8_QK:
                                for k in range(KD):
                                    nc.tensor.matmul(
                                        ps[:, sl],
                                        wqk_w[:, k, off + m * P:off + (m + 1) * P],
                                        h8[:, k, sl],
                                        start=(k == 0), stop=(k == KD - 1))
                            else:
                                for k in range(KD):
                                    nc.tensor.matmul(
                                        ps[:, sl],
                                        wqkv_w[:, k, off + m * P:off + (m + 1) * P],
                                        h_sb[:, k, sl],
                                        start=(k == 0), stop=(k == KD - 1))
                        if which == 0:
                            nc.scalar.copy(dst[:, m, :], ps[:])
                        else:
                            nc.vector.tensor_copy(dst[:, m, :], ps[:])
                # fp8 dual-row ldweights requires even free sizes/offsets, so
                # the fp8 V carries two ones columns (DH+2) instead of one
                VA = DH + 2 if FP8_AV else DH + 1
                v_aug = ep.tile([P, SCH, H, VA], f8 if FP8_AV else bf16,
                                tag="vaug")
                nc.vector.memset(v_aug[:], 1.0)
                for tch in range(0, SCH, 2):
                    ps = pwide()
                    for j in range(2):
                        half = slice(j * 512, (j + 1) * 512)
                        tsl = slice((tch + j) * P, (tch + j + 1) * P)
                        for k in range(KD):
                            if FP8_QK:
                                nc.tensor.matmul(ps[:, half], h_sb[:, k, tsl],
                                                 wv_w[:, k, :],
                                                 start=(k == 0), stop=(k == KD - 1))
                            else:
                                nc.tensor.matmul(ps[:, half], h_sb[:, k, tsl],
                                                 wqkv_w[:, k, 2 * D:3 * D],
                                                 start=(k == 0), stop=(k == KD - 1))
                    nc.vector.tensor_copy(
                        v_aug[:, tch:tch + 2, :, 0:DH],
                        ps[:].rearrange("p (c h d) -> p c h d", c=2, d=DH))
                oT = ep.tile([P, KD, S], bf16, tag="oT")
                for ko in range(KD):
                    if FP8_AV:
                        # et pairs: [P, 2, S] fp8, pair dim = token chunks
                        etp = [[None] * 4, [None] * 4]
                        for tchp in range(4):
                            for hp in range(2):
                                p0 = DH * hp
                                et = ep.tile([P, 2, S], f8, tag="expT", bufs=8)
                                for i in range(2):
                                    tch = 2 * tchp + i
                                    ps = pwide()
                                    for sc in range(SC):
                                        sl = slice(sc * 512, (sc + 1) * 512)
                                        nc.tensor.matmul(
                                            ps[:, sl],
                                            kT[p0:p0 + DH, ko, tch * P:(tch + 1) * P],
                                            qT[p0:p0 + DH, ko, sl],
                                            start=True, stop=True)
                                    nc.scalar.activation(et[:, i, :], ps[:], AF.Exp)
                                etp[hp][tchp] = et
                        for hp in range(2):
                            h = 2 * ko + hp
                            p0 = DH * hp
                            zos = [psmall() for _ in range(SC)]
                            for tchp in range(4):
                                for sc in range(SC):
                                    sl = slice(sc * 512, (sc + 1) * 512)
                                    nc.tensor.matmul(
                                        zos[sc][:VA, :],
                                        v_aug[:, 2 * tchp:2 * tchp + 2, h, :],
                                        etp[hp][tchp][:, :, sl], perf_mode=DR,
                                        start=(tchp == 0), stop=(tchp == 3))
                            for sc in range(SC):
                                sl = slice(sc * 512, (sc + 1) * 512)
                                zo = zos[sc]
                                rcp = ep.tile([1, 512], f32, tag="rcp", bufs=4)
                                nc.vector.reciprocal(rcp[:], zo[DH:DH + 1, :])
                                bc = psmall()
                                nc.tensor.matmul(bc[:DH, :], ones_r[:, :DH], rcp[:],
                                                 start=True, stop=True)
                                bcs = ep.tile([DH, 512], bf16, tag="bcs", bufs=4)
                                nc.scalar.copy(bcs[:], bc[:DH, :])
                                nc.vector.tensor_mul(oT[p0:p0 + DH, ko, sl],
                                                     zo[0:DH, :], bcs[:])
                    else:
                        ets = [[], []]
                        for tch in range(SCH):
                            for hp in range(2):
                                p0 = DH * hp
                                ps = pwide()
                                for sc in range(SC):
                                    sl = slice(sc * 512, (sc + 1) * 512)
                                    nc.tensor.matmul(
                                        ps[:, sl],
                                        kT[p0:p0 + DH, ko, tch * P:(tch + 1) * P],
                                        qT[p0:p0 + DH, ko, sl], start=True, stop=True)
                                et = ep.tile([P, 1024], bf16, tag="expT", bufs=8)
                                nc.scalar.activation(et[:], ps[:], AF.Exp)
                                ets[hp].append(et)
                        for hp in range(2):
                            h = 2 * ko + hp
                            p0 = DH * hp
                            zos = [psmall() for _ in range(SC)]
                            for tch in range(SCH):
                                for sc in range(SC):
                                    sl = slice(sc * 512, (sc + 1) * 512)
                                    nc.tensor.matmul(zos[sc][:DH + 1, :],
                                                     v_aug[:, tch, h, :],
                                                     ets[hp][tch][:, sl],
                                                     start=(tch == 0),
                                                     stop=(tch == SCH - 1))
                            for sc in range(SC):
                                sl = slice(sc * 512, (sc + 1) * 512)
                                zo = zos[sc]
                                rcp = ep.tile([1, 512], f32, tag="rcp", bufs=4)
                                nc.vector.reciprocal(rcp[:], zo[DH:DH + 1, :])
                                bc = psmall()
                                nc.tensor.matmul(bc[:DH, :], ones_r[:, :DH], rcp[:],
                                                 start=True, stop=True)
                                bcs = ep.tile([DH, 512], bf16, tag="bcs", bufs=4)
                                nc.scalar.copy(bcs[:], bc[:DH, :])
                                nc.vector.tensor_mul(oT[p0:p0 + DH, ko, sl],
                                                     zo[0:DH, :], bcs[:])
                r_t = ep.tile([P, KD, S], bf16, tag="resid")
                rsq_t = ep.tile([P, KD, S], bf16, tag="rsq")
                for m in range(KD):
                    ps = pwide()
                    for k in range(KD):
                        for sc in range(SC):
                            sl = slice(sc * 512, (sc + 1) * 512)
                            nc.tensor.matmul(ps[:, sl], wo_w[:, k, m * P:(m + 1) * P],
                                             oT[:, k, sl],
                                             start=(k == 0), stop=(k == KD - 1))
                    if flags["bo"]:
                        nc.vector.tensor_scalar_add(ps[:], ps[:], bo_c[l][:, m:m + 1])
                    nc.vector.tensor_add(r_t[:, m, :], ps[:], h_sb[:, m, :])
                    nc.gpsimd.tensor_mul(rsq_t[:, m, :], r_t[:, m, :], r_t[:, m, :])
                h_sb = emit_ln(ep, r_t, rsq_t, l, 0)
                if FP8_FFN1:
                    h28 = ep.tile([P, KD, S], f8, tag="h28", bufs=1)
                    for m in range(KD):
                        nc.gpsimd.tensor_copy(h28[:, m, :], h_sb[:, m, :])
                r_t = ep.tile([P, KD, S], bf16, tag="resid")
                rsq_t = ep.tile([P, KD, S], bf16, tag="rsq")
                for sc in range(SC):
                    sl = slice(sc * 512, (sc + 1) * 512)
                    ff = ep.tile([P, NF, 512], f8 if FP8_FFN2 else bf16,
                                 tag="ffT", bufs=2)
                    for m in range(0, NF, 2):
                        ps = pwide()
                        for j in range(2):
                            half = slice(j * 512, (j + 1) * 512)
                            if FP8_FFN1:
                                for k in range(0, KD, 2):
                                    nc.tensor.matmul(
                                        ps[:, half],
                                        w1_w[:, k:k + 2, (m + j) * P:(m + j + 1) * P],
                                        h28[:, k:k + 2, sl], perf_mode=DR,
                                        start=(k == 0), stop=(k == KD - 2))
                            else:
                                for k in range(KD):
                                    nc.tensor.matmul(
                                        ps[:, half],
                                        w1_w[:, k, (m + j) * P:(m + j + 1) * P],
                                        h_sb[:, k, sl],
                                        start=(k == 0), stop=(k == KD - 1))
                        psv = ps[:].rearrange("p (c s) -> p c s", c=2)
                        if flags["b1"]:
                            nc.vector.tensor_scalar(ff[:, m:m + 2, :], psv,
                                                    b1_c[l][:, m:m + 1], 0.0,
                                                    OP.add, OP.max)
                        elif (m // 2) % 2 == 0:
                            nc.scalar.activation(ff[:, m:m + 2, :], psv, AF.Relu)
                        else:
                            nc.vector.tensor_scalar(ff[:, m:m + 2, :], psv,
                                                    0.0, None, OP.max)
                    for m in range(0, KD, 2):
                        ps = pwide()
                        for j in range(2):
                            half = slice(j * 512, (j + 1) * 512)
                            if FP8_FFN2:
                                for k in range(0, NF, 2):
                                    nc.tensor.matmul(
                                        ps[:, half],
                                        w2_w[:, k:k + 2, (m + j) * P:(m + j + 1) * P],
                                        ff[:, k:k + 2, :], perf_mode=DR,
                                        start=(k == 0), stop=(k == NF - 2))
                            else:
                                for k in range(NF):
                                    nc.tensor.matmul(
                                        ps[:, half],
                                        w2_w[:, k, (m + j) * P:(m + j + 1) * P],
                                        ff[:, k, :],
                                        start=(k == 0), stop=(k == NF - 1))
                        psv = ps[:].rearrange("p (c s) -> p c s", c=2)
                        if flags["b2"]:
                            nc.vector.tensor_scalar_add(psv, psv, b2_c[l][:, m:m + 1])
                        nc.vector.tensor_add(r_t[:, m:m + 2, sl], psv,
                                             h_sb[:, m:m + 2, sl])
                        nc.gpsimd.tensor_mul(rsq_t[:, m:m + 2, sl],
                                             r_t[:, m:m + 2, sl],
                                             r_t[:, m:m + 2, sl])
                return emit_ln(ep, r_t, rsq_t, l, 2)

            def router_phase(rp, h_sb):
                rtq_w = wload(rp, rtq_d, D, D, bf16, "rtq", nsplit=2)
                wout_w = wload(rp, wout_d, D, IN_DIM, bf16, "wout", nsplit=2)
                qrT = rp.tile([P, KD, S], bf16, tag="qrT")
                for m in range(KD):
                    ps = pwide()
                    for k in range(KD):
                        for sc in range(SC):
                            sl = slice(sc * 512, (sc + 1) * 512)
                            nc.tensor.matmul(ps[:, sl], rtq_w[:, k, m * P:(m + 1) * P],
                                             h_sb[:, k, sl],
                                             start=(k == 0), stop=(k == KD - 1))
                    nc.vector.tensor_copy(qrT[:, m, :], ps[:])
                pk = rp.tile([P, 16], f32, tag="pk")
                for sch in range(SCH):
                    ps = pmm()
                    for k in range(KD):
                        nc.tensor.matmul(ps[:, :M], qrT[:, k, sch * P:(sch + 1) * P],
                                         phiT_b[:, k, :],
                                         start=(k == 0), stop=(k == KD - 1))
                    lg = rp.tile([P, M], f32, tag="lgtok", bufs=2)
                    nc.vector.tensor_copy(lg[:], ps[:, :M])
                    mx = rp.tile([P, 8], f32, tag="mx8", bufs=2)
                    nc.vector.max(mx[:], lg[:])
                    e4 = rp.tile([P, 4], f32, tag="e4", bufs=2)
                    nc.vector.tensor_scalar(e4[:], mx[:, 0:4], mx[:, 0:1], None,
                                            OP.subtract)
                    nc.scalar.activation(e4[:], e4[:], AF.Exp)
                    s4 = rp.tile([P, 1], f32, tag="s4", bufs=2)
                    nc.vector.reduce_sum(s4[:], e4[:], axis=mybir.AxisListType.X)
                    nc.scalar.activation(s4[:], s4[:], AF.Ln)
                    nc.vector.tensor_add(s4[:], s4[:], mx[:, 0:1])
                    nc.vector.tensor_scalar_mul(pk[:, 2 * sch:2 * sch + 1], s4[:], -1.0)
                    mid = rp.tile([P, 1], f32, tag="mid", bufs=2)
                    nc.vector.tensor_add(mid[:], mx[:, 3:4], mx[:, 4:5])
                    nc.vector.tensor_scalar_mul(pk[:, 2 * sch + 1:2 * sch + 2],
                                                mid[:], 0.5)
                tp_ps = psmall()
                nc.tensor.transpose(tp_ps[:16, :P], pk[:], ident[:])
                t16 = rp.tile([16, P], f32, tag="t16sb")
                nc.vector.tensor_copy(t16[:], tp_ps[:16, :P])
                dsc = dpool.tile([16, P], f32, tag="dscr")
                nc.sync.dma_start(dsc[:], t16[:])
                brow = rp.tile([1, S], f32, tag="brow")
                mrow = rp.tile([1, S], f32, tag="mrow")
                for sch in range(SCH):
                    nc.sync.dma_start(brow[:, sch * P:(sch + 1) * P],
                                      dsc[2 * sch:2 * sch + 1, :])
                    nc.sync.dma_start(mrow[:, sch * P:(sch + 1) * P],
                                      dsc[2 * sch + 1:2 * sch + 2, :])
                bias_b = rp.tile([P, S], bf16, tag="biasb")
                mid_b = rp.tile([P, S], bf16, tag="midb")
                for sc in range(SC):
                    sl = slice(sc * 512, (sc + 1) * 512)
                    ps = pmm()
                    nc.tensor.matmul(ps[:], ones_r[:, :P], brow[:, sl],
                                     start=True, stop=True)
                    nc.vector.tensor_copy(bias_b[:, sl], ps[:])
                    ps2 = pmm()
                    nc.tensor.matmul(ps2[:], ones_r[:, :P], mrow[:, sl],
                                     start=True, stop=True)
                    nc.vector.tensor_copy(mid_b[:, sl], ps2[:])
                pt = rp.tile([P, 2, S], bf16, tag="PT")
                for mch in range(2):
                    for sc in range(SC):
                        sl = slice(sc * 512, (sc + 1) * 512)
                        ps = pmm()
                        for k in range(KD):
                            nc.tensor.matmul(ps[:], phiT_b[:, k, mch * P:(mch + 1) * P],
                                             qrT[:, k, sl],
                                             start=(k == 0), stop=(k == KD - 1))
                        t1 = rp.tile([P, 512], f32, tag="ptt1", bufs=2)
                        nc.vector.tensor_add(t1[:], ps[:], bias_b[:, sl])
                        eb = rp.tile([P, 512], bf16, tag="pte", bufs=2)
                        nc.scalar.activation(eb[:], t1[:], AF.Exp)
                        gb = rp.tile([P, 512], bf16, tag="ptg", bufs=2)
                        nc.vector.tensor_tensor(gb[:], ps[:], mid_b[:, sl],
                                                op=OP.is_ge)
                        nc.gpsimd.tensor_mul(pt[:, mch, sl], eb[:], gb[:])
                routed = rp.tile([P, KD, S], bf16, tag="routedT")
                for m in range(KD):
                    ps = pwide()
                    for k in range(2):
                        for sc in range(SC):
                            sl = slice(sc * 512, (sc + 1) * 512)
                            nc.tensor.matmul(ps[:, sl], z_sb[:, k, m * P:(m + 1) * P],
                                             pt[:, k, sl],
                                             start=(k == 0), stop=(k == 1))
                    nc.vector.tensor_add(routed[:, m, :], ps[:], h_sb[:, m, :])
                out_sb = rp.tile([P, 2, S], f32, tag="outT")
                for m in range(2):
                    ps = pwide()
                    for k in range(KD):
                        for sc in range(SC):
                            sl = slice(sc * 512, (sc + 1) * 512)
                            nc.tensor.matmul(ps[:, sl], wout_w[:, k, m * P:(m + 1) * P],
                                             routed[:, k, sl],
                                             start=(k == 0), stop=(k == KD - 1))
                    if flags["bout"]:
                        nc.vector.tensor_scalar_add(out_sb[:, m, :], ps[:],
                                                    bout_c[:, m:m + 1])
                    else:
                        nc.vector.tensor_copy(out_sb[:, m, :], ps[:])
                if reps == 1:
                    nc.sync.dma_start(out_d.rearrange("(o p) s -> p o s", p=P),
                                      out_sb[:])
                else:
                    # timing builds: accumulate so repeated bodies stay live
                    nc.gpsimd.dma_start(out_d.rearrange("(o p) s -> p o s", p=P),
                                        out_sb[:], accum_op=OP.add)

            def body():
                with tc.tile_pool(name="inproj", bufs=1) as ip:
                    h_sb = inproj_phase(ip)
                with tc.tile_pool(name="bank", bufs=1) as bp:
                    bank_phase(bp)
                with tc.tile_pool(name="enc", bufs=1) as ep:
                    for l in range(L):
                        h_sb = encoder_layer(ep, l, h_sb)
                with tc.tile_pool(name="router", bufs=1) as rp:
                    router_phase(rp, h_sb)

            for _ in range(reps):
                body()

    # Pin every activation to the one table set containing Exp+Ln+Relu so
    # the table-load pass emits a single load instead of thrashing between
    # exp_and_others and natural_log (~2.7us per switch).
    import concourse.bacc as bacc_mod
    import concourse.hw_specs as hw_specs_mod
    orig = bacc_mod.get_activation_tables
    keepset = "natural_log_exp_and_others"

    def pinned(arch):
        return {k: (v if k == keepset else set())
                for k, v in hw_specs_mod.get_activation_tables(arch).items()}

    bacc_mod.get_activation_tables = pinned
    try:
        nc.compile()
    finally:
        bacc_mod.get_activation_tables = orig
    return nc


def _flags_from(inputs):
    nz = lambda a: bool(np.any(np.asarray(a)))
    return {
        "bqkv": nz(inputs["enc_bqkv"]),
        "bo": nz(inputs["enc_bo"]),
        "b1": nz(inputs["ff_b1"]),
        "b2": nz(inputs["ff_b2"]),
        "ln": (nz(inputs["ln1_b"]) or nz(inputs["ln2_b"])
               or nz(np.asarray(inputs["ln1_g"]) - 1.0)
               or nz(np.asarray(inputs["ln2_g"]) - 1.0)),
        "bout": nz(inputs["b_out"]),
        "mask": not bool(np.all(np.asarray(inputs["mask"]))),
    }


def _pe_table():
    pos = np.arange(S, dtype=np.float32)[:, None]
    div = np.exp(np.arange(0, D, 2, dtype=np.float32) * (-np.log(10000.0) / D))
    pe = np.zeros((S, D), np.float32)
    pe[:, 0::2] = np.sin(pos * div)
    pe[:, 1::2] = np.cos(pos * div)
    return pe


def make_in_maps(inputs):
    f = np.float32
    b16 = ml_dtypes.bfloat16
    f8t = ml_dtypes.float8_e4m3
    a = {k: np.asarray(v) for k, v in inputs.items()}
    flags = _flags_from(a)
    pe = _pe_table()

    wqkv = a["enc_Wqkv"].astype(f).copy()
    wqkv[:, :, 0:D] *= 1.0 / np.sqrt(DH)          # fold q scale
    sak = a["sa_Wk"].astype(f) * (BETA * ETA / np.sqrt(DH))
    rtq = a["rt_Wq"].astype(f) * (1.0 / np.sqrt(D))

    shared = {
        "win": a["Win"].astype(b16), "wout": a["Wout"].astype(b16),
        "wo4": a["enc_Wo"].astype(b16),
        "saq": a["sa_Wq"].astype(b16), "sak": sak.astype(b16),
        "sav": a["sa_Wv"].astype(b16), "sao": a["sa_Wo"].astype(b16),
        "rtq": rtq.astype(b16),
    }
    if FP8_QK:
        shared["wqk8"] = np.ascontiguousarray(wqkv[:, :, 0:2 * D]).astype(f8t)
        shared["wvb"] = np.ascontiguousarray(wqkv[:, :, 2 * D:3 * D]).astype(b16)
    else:
        shared["wqkv"] = wqkv.astype(b16)
    shared["w18"] = a["ff_W1"].astype(f8t if FP8_FFN1 else b16)
    shared["w28"] = a["ff_W2"].astype(f8t if FP8_FFN2 else b16)
    if flags["bo"]:
        shared["bo4"] = a["enc_bo"].astype(f)
    if flags["b1"]:
        shared["b14"] = a["ff_b1"].astype(f)
    if flags["b2"]:
        shared["b24"] = a["ff_b2"].astype(f)
    if flags["ln"]:
        shared["lnp"] = np.stack(
            [a["ln1_g"], a["ln1_b"], a["ln2_g"], a["ln2_b"]], axis=1).astype(f)
    if flags["bout"]:
        shared["bout"] = a["b_out"].astype(f)

    maps = []
    for b in range(B):
        m = dict(shared)
        m["xT"] = np.ascontiguousarray(a["x_t"][b].T).astype(b16)
        m["pep"] = np.ascontiguousarray(
            pe.T + (a["t_embed"][b] + a["b_in"])[:, None]).astype(b16)
        phi = a["Phi"][b].astype(f)
        m["phiT"] = np.ascontiguousarray(phi.T).astype(b16)
        m["phi2c"] = np.ascontiguousarray(2.0 * C2 * phi.T).astype(b16)
        p2 = np.sum(phi * phi, axis=-1)                       # [M]
        sig = np.mean(a["Sig"][b].astype(f) ** 2, axis=-1)    # [M]
        drow = (GAMMA * np.log(a["Size"][b].astype(f) + 1e-6)
                - 0.5 * sig - C2 * p2)
        m["drow"] = drow.reshape(1, M).astype(f)
        m["p2m"] = (-C2 * p2).reshape(1, M).astype(f)
        maps.append(m)
    return maps, flags


def get_nc(flags, reps=1):
    key = (tuple(sorted(flags.items())), reps)
    if key not in _CACHE:
        _CACHE[key] = _build(flags, reps)
    return _CACHE[key]


def kernel(**inputs):
    from concourse.bass_utils import run_bass_kernel_spmd
    maps, flags = make_in_maps(inputs)
    nc = get_nc(flags, reps=1)
    res = run_bass_kernel_spmd(nc, maps, list(range(B)))
    out = np.stack([np.ascontiguousarray(res.results[b]["outT"].T)
                    for b in range(B)])
    return out.astype(np.float32)


# revision 27
# speedup vs baseline: 1.1196x; 1.1196x over previous
"""Trainium2 Bass kernel for nn_BankedDenoiser.

Data-parallel over batch: 8 batch elements -> 8 NeuronCores, one element per
core, no collectives. Activations are kept feature-major (hT [D, S]) in SBUF so
every y = x @ W becomes matmul(lhsT=W_chunk, rhs=hT) with weights in their
natural DRAM layout. Matmuls run in bf16 (fp8e4m3 DoubleRow where flagged)
with fp32 PSUM accumulation. All weights are converted to bf16/fp8 on the
host and DMA'd directly (no on-chip staging), with scalar factors
(1/sqrt(DH), BETA*ETA/sqrt(DH), 1/sqrt(D)) folded into the host weights, and
the positional encoding + t_embed + b_in folded into one host tensor.

Attention: scores are produced transposed (scoresT [key, query]) so the
exponentiated weights feed the A@V matmul as rhs; the softmax denominator
comes from augmenting token-major V with a ones column, and per-query
normalization is applied via a rank-1 ones-broadcast matmul of the
reciprocal row. exp() runs without max-subtraction (scores are bounded for
this module's weight scale) straight out of PSUM on ScalarE. A@V uses
fp8e4m3 DoubleRow over token-chunk pairs (exp weights quantize harmlessly).

LayerNorm (feature-major): sum / sum-of-squares via ones-column matmuls on
the PE, rstd = exp(-0.5 * ln(var/D + eps)); mean/rstd rows broadcast with
rank-1 matmuls. Squares and LN-apply multiplies run on Pool (gpsimd) to keep
ScalarE free for exp.

Bank-attention key biases (gamma*ln(Size)-sig/2-c*|phi|^2) are tiny [M]
vectors computed on the host.

Router top-4: Max8 gives each token's top-8 logits sorted; the dense
dispatch matrix P^T[m, s] = exp(logit - logsumexp_top4) * (logit >= midgap)
is built transposed and applied as a matmul against token-major Z.
"""

import numpy as np
import ml_dtypes

B, S, IN_DIM, D, H, L, M, TOPK = 8, 1024, 256, 512, 8, 4, 256, 4
DFF = 2048
DH = D // H
TAU, GAMMA, BETA, ETA = 1.0, 0.3, 1.0, 1.0
P = 128
KD = D // P          # 4 feature chunks of 128
SC = S // 512        # 2 column chunks of 512
SCH = S // P         # 8 token chunks of 128
NF = DFF // P        # 16 dff chunks
C2 = BETA / (TAU * D)

# fp8e4m3 DoubleRow toggles per site
FP8_QK = False        # q,k projections from fp8 h
FP8_QK_DR = False    # use DoubleRow for the q,k matmuls
FP8_AV = False        # attention A@V with fp8 exp-weights and V
FP8_FFN1 = False      # ffn W1 matmul from fp8 h
FP8_FFN2 = False      # ffn W2 matmul from fp8 relu output

_CACHE = {}


def _build(flags, reps=1):
    import concourse.tile as tile
    from concourse import bacc, mybir
    from concourse.masks import make_identity

    f32 = mybir.dt.float32
    bf16 = mybir.dt.bfloat16
    f8 = mybir.dt.float8e4
    AF = mybir.ActivationFunctionType
    OP = mybir.AluOpType
    DR = mybir.MatmulPerfMode.DoubleRow

    assert not flags["mask"], "non-trivial mask not supported"
    assert not flags["bqkv"], "nonzero enc qkv bias not supported"

    nc = bacc.Bacc("TRN2", target_bir_lowering=False, debug=False, num_devices=8)

    def din(name, shape, dt=bf16):
        return nc.dram_tensor(name, shape, dt, kind="ExternalInput").ap()

    xT_d = din("xT", [IN_DIM, S])
    pep_d = din("pep", [D, S])          # peT + t_embed[b] + b_in (host-folded)
    phiT_d = din("phiT", [D, M])
    phi2c_d = din("phi2c", [D, M])      # 2*C2*phiT
    drow_d = din("drow", [1, M], f32)   # key bias row (host-computed)
    p2m_d = din("p2m", [1, M], f32)     # -C2*|phi|^2 row (host-computed)
    win_d = din("win", [IN_DIM, D])
    if FP8_QK:
        wqk_d = din("wqk8", [L, D, 2 * D], f8)   # q cols pre-scaled 1/sqrt(DH)
        wv_d = din("wvb", [L, D, D])
    else:
        wqkv_d = din("wqkv", [L, D, 3 * D])
    wo_d = din("wo4", [L, D, D])
    w1_d = din("w18", [L, D, DFF], f8 if FP8_FFN1 else bf16)
    w2_d = din("w28", [L, DFF, D], f8 if FP8_FFN2 else bf16)
    saq_d = din("saq", [D, D])
    sak_d = din("sak", [D, D])          # pre-scaled BETA*ETA/sqrt(DH)
    sav_d = din("sav", [D, D])
    sao_d = din("sao", [D, D])
    rtq_d = din("rtq", [D, D])          # pre-scaled 1/sqrt(D)
    wout_d = din("wout", [D, IN_DIM])
    bo_d = din("bo4", [L, D], f32) if flags["bo"] else None
    b1_d = din("b14", [L, DFF], f32) if flags["b1"] else None
    b2_d = din("b24", [L, D], f32) if flags["b2"] else None
    ln_d = din("lnp", [L, 4, D], f32) if flags["ln"] else None
    bout_d = din("bout", [IN_DIM], f32) if flags["bout"] else None
    out_d = nc.dram_tensor("outT", [IN_DIM, S], f32, kind="ExternalOutput").ap()

    with tile.TileContext(nc) as tc:
        with (tc.tile_pool(name="const", bufs=1) as cpool,
              tc.tile_pool(name="keep", bufs=1) as keep,
              tc.tile_pool(name="pswide", bufs=2, space="PSUM") as ps_wide,
              tc.tile_pool(name="psmm", bufs=1, space="PSUM") as ps_mm,
              tc.tile_pool(name="pssm", bufs=1, space="PSUM") as ps_sm,
              tc.tile_pool(name="dram", bufs=2, space="DRAM") as dpool):

            ident = cpool.tile([P, P], f32)
            make_identity(nc, ident[:])
            ones_r = cpool.tile([1, D], f32)
            nc.vector.memset(ones_r[:], 1.0)
            ones_cb = cpool.tile([P, 1], bf16)
            nc.vector.memset(ones_cb[:], 1.0)
            eps5_r = cpool.tile([1, 1], f32)
            nc.vector.memset(eps5_r[:], 1e-5)

            _n = [0]

            def pwide():
                _n[0] += 1
                return ps_wide.tile([P, 1024], f32, tag="wide", bufs=2,
                                    name=f"wps{_n[0]}")

            def pmm():
                _n[0] += 1
                return ps_mm.tile([P, 512], f32, tag="mm", bufs=1,
                                  name=f"mmps{_n[0]}")

            def psmall():
                _n[0] += 1
                return ps_sm.tile([P, 512], f32, tag="small", bufs=3,
                                  name=f"sps{_n[0]}")

            # DRAM [K, N] -> SBUF [P, K//P, N], split into nsplit DMAs
            def wload(pool, dram2d, K, N, dt, tag, bufs=1, nsplit=4):
                ko_n = K // P
                w = pool.tile([P, ko_n, N], dt, tag=tag, bufs=bufs)
                src = dram2d.rearrange("(ko p) m -> p ko m", p=P)
                step = max(1, ko_n // nsplit)
                for c0 in range(0, ko_n, step):
                    cn = min(step, ko_n - c0)
                    nc.sync.dma_start(w[:, c0:c0 + cn, :], src[:, c0:c0 + cn, :])
                return w

            def col_from(dram1d, n, tag):
                t = cpool.tile([P, n // P], f32, tag=tag)
                nc.sync.dma_start(t[:], dram1d.rearrange("(o p) -> p o", p=P))
                return t

            if flags["bo"]:
                bo_c = [col_from(bo_d[l], D, f"bo{l}") for l in range(L)]
            if flags["b1"]:
                b1_c = [col_from(b1_d[l], DFF, f"b1{l}") for l in range(L)]
            if flags["b2"]:
                b2_c = [col_from(b2_d[l], D, f"b2{l}") for l in range(L)]
            if flags["ln"]:
                ln_c = [[col_from(ln_d[l, j], D, f"ln{l}_{j}") for j in range(4)]
                        for l in range(L)]
            if flags["bout"]:
                bout_c = col_from(bout_d, IN_DIM, "boutc")

            # persistent across phases
            phiT_b = keep.tile([P, KD, M], bf16, tag="phiTb")
            nc.sync.dma_start(phiT_b[:], phiT_d.rearrange("(ko p) m -> p ko m", p=P))
            z_sb = keep.tile([P, 2, D], bf16, tag="ztok")

            def inproj_phase(ip):
                win_w = wload(ip, win_d, IN_DIM, D, bf16, "win", nsplit=2)
                xT_b = ip.tile([P, 2, S], bf16, tag="xTb")
                nc.sync.dma_start(xT_b[:], xT_d.rearrange("(ko p) s -> p ko s", p=P))
                pep_sb = ip.tile([P, KD, S], bf16, tag="pep")
                nc.sync.dma_start(pep_sb[:, 0:2, :],
                                  pep_d.rearrange("(ko p) s -> p ko s", p=P)[:, 0:2, :])
                nc.sync.dma_start(pep_sb[:, 2:4, :],
                                  pep_d.rearrange("(ko p) s -> p ko s", p=P)[:, 2:4, :])
                h_sb = keep.tile([P, KD, S], bf16, tag="hT", bufs=2)
                for m in range(KD):
                    ps = pwide()
                    for k in range(2):
                        for sc in range(SC):
                            sl = slice(sc * 512, (sc + 1) * 512)
                            nc.tensor.matmul(ps[:, sl], win_w[:, k, m * P:(m + 1) * P],
                                             xT_b[:, k, sl],
                                             start=(k == 0), stop=(k == 1))
                    nc.vector.tensor_add(h_sb[:, m, :], ps[:], pep_sb[:, m, :])
                return h_sb

            def bank_phase(bp):
                saq_w = wload(bp, saq_d, D, D, bf16, "sab", bufs=1, nsplit=2)
                sak_w = wload(bp, sak_d, D, D, bf16, "sab", bufs=1, nsplit=2)
                sav_w = wload(bp, sav_d, D, D, bf16, "sab", bufs=1, nsplit=2)
                sao_w = wload(bp, sao_d, D, D, bf16, "sab", bufs=1, nsplit=2)
                phi2c_b = bp.tile([P, KD, M], bf16, tag="phi2c")
                nc.sync.dma_start(phi2c_b[:],
                                  phi2c_d.rearrange("(ko p) m -> p ko m", p=P))
                d_row = bp.tile([1, M], f32, tag="drow")
                nc.sync.dma_start(d_row[:], drow_d)
                p2m_row = bp.tile([1, M], f32, tag="p2mrow")
                nc.sync.dma_start(p2m_row[:], p2m_d)

                qTb = bp.tile([P, KD, M], bf16, tag="qTb")
                kTb = bp.tile([P, KD, M], bf16, tag="kTb")
                for dst, wmat in ((qTb, saq_w), (kTb, sak_w)):
                    for m in range(KD):
                        ps = pmm()
                        for k in range(KD):
                            nc.tensor.matmul(ps[:, :M],
                                             wmat[:, k, m * P:(m + 1) * P],
                                             phiT_b[:, k, :],
                                             start=(k == 0), stop=(k == KD - 1))
                        nc.vector.tensor_copy(dst[:, m, :], ps[:, :M])
                vb_aug = bp.tile([P, 2, H, DH + 1], bf16, tag="vbaug")
                nc.vector.memset(vb_aug[:], 1.0)
                for nch in range(2):
                    ps = pmm()
                    for k in range(KD):
                        nc.tensor.matmul(ps[:], phiT_b[:, k, nch * P:(nch + 1) * P],
                                         sav_w[:, k, :],
                                         start=(k == 0), stop=(k == KD - 1))
                    nc.vector.tensor_copy(
                        vb_aug[:, nch, :, 0:DH],
                        ps[:].rearrange("p (h c) -> p h c", c=DH))

                oTb = bp.tile([P, KD, M], bf16, tag="oTb")
                for h in range(H):
                    p0, ko = DH * (h % 2), h // 2
                    eb = bp.tile([P, 2, M], bf16, tag="expb", bufs=2)
                    for nch in range(2):
                        ps = pmm()
                        for k in range(KD):
                            nc.tensor.matmul(ps[:, :M],
                                             phiT_b[:, k, nch * P:(nch + 1) * P],
                                             phi2c_b[:, k, :],
                                             start=(k == 0), stop=False)
                        nc.tensor.matmul(ps[:, :M], d_row[:, nch * P:(nch + 1) * P],
                                         ones_r[:, :M], start=False, stop=False)
                        nc.tensor.matmul(ps[:, :M], ones_r[:, :P], p2m_row[:],
                                         start=False, stop=False)
                        nc.tensor.matmul(ps[:, :M],
                                         kTb[p0:p0 + DH, ko, nch * P:(nch + 1) * P],
                                         qTb[p0:p0 + DH, ko, :],
                                         start=False, stop=True)
                        nc.scalar.activation(eb[:, nch, :], ps[:, :M], AF.Exp)
                    zb = psmall()
                    for nch in range(2):
                        nc.tensor.matmul(zb[:DH + 1, :M], vb_aug[:, nch, h, :],
                                         eb[:, nch, :],
                                         start=(nch == 0), stop=(nch == 1))
                    rb = bp.tile([1, M], f32, tag="recb", bufs=2)
                    nc.vector.reciprocal(rb[:], zb[DH:DH + 1, :M])
                    bc = psmall()
                    nc.tensor.matmul(bc[:DH, :M], ones_r[:, :DH], rb[:],
                                     start=True, stop=True)
                    bcs = bp.tile([DH, M], bf16, tag="bcsb", bufs=2)
                    nc.scalar.copy(bcs[:], bc[:DH, :M])
                    nc.vector.tensor_mul(oTb[p0:p0 + DH, ko, :], zb[0:DH, :M], bcs[:])
                for mch in range(2):
                    ps = pmm()
                    for k in range(KD):
                        nc.tensor.matmul(ps[:], oTb[:, k, mch * P:(mch + 1) * P],
                                         sao_w[:, k, :],
                                         start=(k == 0), stop=(k == KD - 1))
                    nc.vector.tensor_copy(z_sb[:, mch, :], ps[:])

            def emit_ln(ep, r_t, rsq_t, lidx, lnoff):
                # r_t: bf16 [P, KD, S]; rsq_t: bf16 [P, KD, S] (squares)
                out = keep.tile([P, KD, S], bf16, tag="hT", bufs=2)
                for sc in range(SC):
                    sl = slice(sc * 512, (sc + 1) * 512)
                    ps1 = psmall()
                    for k in range(KD):
                        nc.tensor.matmul(ps1[:1, :], ones_cb[:], r_t[:, k, sl],
                                         start=(k == 0), stop=(k == KD - 1))
                    ps2 = psmall()
                    for k in range(KD):
                        nc.tensor.matmul(ps2[:1, :], ones_cb[:], rsq_t[:, k, sl],
                                         start=(k == 0), stop=(k == KD - 1))
                    mu_row = ep.tile([1, 512], f32, tag="murow", bufs=2)
                    nc.vector.tensor_scalar_mul(mu_row[:], ps1[:1, :], 1.0 / D)
                    tr = ep.tile([1, 512], f32, tag="tmprow", bufs=2)
                    nc.vector.tensor_mul(tr[:], ps1[:1, :], mu_row[:])
                    var_row = ep.tile([1, 512], f32, tag="varrow", bufs=2)
                    nc.vector.tensor_sub(var_row[:], ps2[:1, :], tr[:])
                    nc.scalar.activation(var_row[:], var_row[:], AF.Ln,
                                         bias=eps5_r[:], scale=1.0 / D)
                    rstd_row = ep.tile([1, 512], f32, tag="rstdrow", bufs=2)
                    nc.scalar.activation(rstd_row[:], var_row[:], AF.Exp, scale=-0.5)
                    mb_ps = psmall()
                    nc.tensor.matmul(mb_ps[:], ones_r[:, :P], mu_row[:],
                                     start=True, stop=True)
                    mb = ep.tile([P, 512], bf16, tag="mubc", bufs=2)
                    nc.vector.tensor_copy(mb[:], mb_ps[:])
                    rb_ps = psmall()
                    nc.tensor.matmul(rb_ps[:], ones_r[:, :P], rstd_row[:],
                                     start=True, stop=True)
                    rbt = ep.tile([P, 512], bf16, tag="rstdbc", bufs=2)
                    nc.vector.tensor_copy(rbt[:], rb_ps[:])
                    for k in range(KD):
                        t1 = ep.tile([P, 512], bf16, tag="lnt1", bufs=4)
                        nc.vector.tensor_sub(t1[:], r_t[:, k, sl], mb[:])
                        if flags["ln"]:
                            t2 = ep.tile([P, 512], bf16, tag="lnt2", bufs=2)
                            nc.gpsimd.tensor_mul(t2[:], t1[:], rbt[:])
                            nc.vector.tensor_scalar(
                                out[:, k, sl], t2[:],
                                ln_c[lidx][lnoff][:, k:k + 1],
                                ln_c[lidx][lnoff + 1][:, k:k + 1], OP.mult, OP.add)
                        else:
                            nc.gpsimd.tensor_mul(out[:, k, sl], t1[:], rbt[:])
                return out

            def encoder_layer(ep, l, h_sb):
                if FP8_QK:
                    wqk_w = wload(ep, wqk_d[l], D, 2 * D, f8, "wqk8", bufs=2,
                                  nsplit=2)
                    wv_w = wload(ep, wv_d[l], D, D, bf16, "wvb", bufs=1, nsplit=2)
                else:
                    wqkv_w = wload(ep, wqkv_d[l], D, 3 * D, bf16, "wqkv", bufs=1,
                                   nsplit=4)
                wo_w = wload(ep, wo_d[l], D, D, bf16, "wo", bufs=1, nsplit=2)
                w1_w = wload(ep, w1_d[l], D, DFF, f8 if FP8_FFN1 else bf16,
                             "w1", bufs=2 if FP8_FFN1 else 1, nsplit=4)
                w2_w = wload(ep, w2_d[l], DFF, D, f8 if FP8_FFN2 else bf16,
                             "w2", bufs=2 if FP8_FFN2 else 1, nsplit=4)
                if FP8_QK or FP8_FFN1:
                    h8 = ep.tile([P, KD, S], f8, tag="h8", bufs=1)
                    for m in range(KD):
                        nc.gpsimd.tensor_copy(h8[:, m, :], h_sb[:, m, :])
                qT = ep.tile([P, KD, S], bf16, tag="qT")
                kT = ep.tile([P, KD, S], bf16, tag="kT")
                for which, dst in ((0, qT), (1, kT)):
                    off = which * D
                    for m in range(KD):
                        ps = pwide()
                        for sc in range(SC):
                            sl = slice(sc * 512, (sc + 1) * 512)
                            if FP8_QK and FP8_QK_DR:
                                for k in range(0, KD, 2):
                                    nc.tensor.matmul(
                                        ps[:, sl],
                                        wqk_w[:, k:k + 2, off + m * P:off + (m + 1) * P],
                                        h8[:, k:k + 2, sl], perf_mode=DR,
                                        start=(k == 0), stop=(k == KD - 2))
                            elif FP8_QK:
                                for k in range(KD):
                                    nc.tensor.matmul(
                                        ps[:, sl],
                                        wqk_w[:, k, off + m * P:off + (m + 1) * P],
                                        h8[:, k, sl],
                                        start=(k == 0), stop=(k == KD - 1))
                            else:
                                for k in range(KD):
                                    nc.tensor.matmul(
                                        ps[:, sl],
                                        wqkv_w[:, k, off + m * P:off + (m + 1) * P],
                                        h_sb[:, k, sl],
                                        start=(k == 0), stop=(k == KD - 1))
                        nc.vector.tensor_copy(dst[:, m, :], ps[:])
                # fp8 dual-row ldweights requires even free sizes/offsets, so
                # the fp8 V carries two ones columns (DH+2) instead of one
                VA = DH + 2 if FP8_AV else DH + 1
                v_aug = ep.tile([P, SCH, H, VA], f8 if FP8_AV else bf16,
                                tag="vaug")
                nc.vector.memset(v_aug[:], 1.0)
                for tch in range(0, SCH, 2):
                    ps = pwide()
                    for j in range(2):
                        half = slice(j * 512, (j + 1) * 512)
                        tsl = slice((tch + j) * P, (tch + j + 1) * P)
                        for k in range(KD):
                            if FP8_QK:
                                nc.tensor.matmul(ps[:, half], h_sb[:, k, tsl],
                                                 wv_w[:, k, :],
                                                 start=(k == 0), stop=(k == KD - 1))
                            else:
                                nc.tensor.matmul(ps[:, half], h_sb[:, k, tsl],
                                                 wqkv_w[:, k, 2 * D:3 * D],
                                                 start=(k == 0), stop=(k == KD - 1))
                    nc.vector.tensor_copy(
                        v_aug[:, tch:tch + 2, :, 0:DH],
                        ps[:].rearrange("p (c h d) -> p c h d", c=2, d=DH))
                oT = ep.tile([P, KD, S], bf16, tag="oT")
                for ko in range(KD):
                    if FP8_AV:
                        # et pairs: [P, 2, S] fp8, pair dim = token chunks
                        etp = [[None] * 4, [None] * 4]
                        for tchp in range(4):
                            for hp in range(2):
                                p0 = DH * hp
                                et = ep.tile([P, 2, S], f8, tag="expT", bufs=8)
                                for i in range(2):
                                    tch = 2 * tchp + i
                                    ps = pwide()
                                    for sc in range(SC):
                                        sl = slice(sc * 512, (sc + 1) * 512)
                                        nc.tensor.matmul(
                                            ps[:, sl],
                                            kT[p0:p0 + DH, ko, tch * P:(tch + 1) * P],
                                            qT[p0:p0 + DH, ko, sl],
                                            start=True, stop=True)
                                    nc.scalar.activation(et[:, i, :], ps[:], AF.Exp)
                                etp[hp][tchp] = et
                        for hp in range(2):
                            h = 2 * ko + hp
                            p0 = DH * hp
                            zos = [psmall() for _ in range(SC)]
                            for tchp in range(4):
                                for sc in range(SC):
                                    sl = slice(sc * 512, (sc + 1) * 512)
                                    nc.tensor.matmul(
                                        zos[sc][:VA, :],
                                        v_aug[:, 2 * tchp:2 * tchp + 2, h, :],
                                        etp[hp][tchp][:, :, sl], perf_mode=DR,
                                        start=(tchp == 0), stop=(tchp == 3))
                            for sc in range(SC):
                                sl = slice(sc * 512, (sc + 1) * 512)
                                zo = zos[sc]
                                rcp = ep.tile([1, 512], f32, tag="rcp", bufs=4)
                                nc.vector.reciprocal(rcp[:], zo[DH:DH + 1, :])
                                bc = psmall()
                                nc.tensor.matmul(bc[:DH, :], ones_r[:, :DH], rcp[:],
                                                 start=True, stop=True)
                                bcs = ep.tile([DH, 512], bf16, tag="bcs", bufs=4)
                                nc.vector.tensor_copy(bcs[:], bc[:DH, :])
                                nc.vector.tensor_mul(oT[p0:p0 + DH, ko, sl],
                                                     zo[0:DH, :], bcs[:])
                    else:
                        ets = [[], []]
                        for tch in range(SCH):
                            for hp in range(2):
                                p0 = DH * hp
                                ps = pwide()
                                for sc in range(SC):
                                    sl = slice(sc * 512, (sc + 1) * 512)
                                    nc.tensor.matmul(
                                        ps[:, sl],
                                        kT[p0:p0 + DH, ko, tch * P:(tch + 1) * P],
                                        qT[p0:p0 + DH, ko, sl], start=True, stop=True)
                                et = ep.tile([P, 1024], bf16, tag="expT", bufs=8)
                                nc.scalar.activation(et[:], ps[:], AF.Exp)
                                ets[hp].append(et)
                        for hp in range(2):
                            h = 2 * ko + hp
                            p0 = DH * hp
                            zos = [psmall() for _ in range(SC)]
                            for tch in range(SCH):
                                for sc in range(SC):
                                    sl = slice(sc * 512, (sc + 1) * 512)
                                    nc.tensor.matmul(zos[sc][:DH + 1, :],
                                                     v_aug[:, tch, h, :],
                                                     ets[hp][tch][:, sl],
                                                     start=(tch == 0),
                                                     stop=(tch == SCH - 1))
                            for sc in range(SC):
                                sl = slice(sc * 512, (sc + 1) * 512)
                                zo = zos[sc]
                                rcp = ep.tile([1, 512], f32, tag="rcp", bufs=4)
                                nc.vector.reciprocal(rcp[:], zo[DH:DH + 1, :])
                                bc = psmall()
                                nc.tensor.matmul(bc[:DH, :], ones_r[:, :DH], rcp[:],
                                                 start=True, stop=True)
                                bcs = ep.tile([DH, 512], bf16, tag="bcs", bufs=4)
                                nc.vector.tensor_copy(bcs[:], bc[:DH, :])
                                nc.vector.tensor_mul(oT[p0:p0 + DH, ko, sl],
                                                     zo[0:DH, :], bcs[:])
                r_t = ep.tile([P, KD, S], bf16, tag="resid")
                rsq_t = ep.tile([P, KD, S], bf16, tag="rsq")
                for m in range(KD):
                    ps = pwide()
                    for k in range(KD):
                        for sc in range(SC):
                            sl = slice(sc * 512, (sc + 1) * 512)
                            nc.tensor.matmul(ps[:, sl], wo_w[:, k, m * P:(m + 1) * P],
                                             oT[:, k, sl],
                                             start=(k == 0), stop=(k == KD - 1))
                    if flags["bo"]:
                        nc.vector.tensor_scalar_add(ps[:], ps[:], bo_c[l][:, m:m + 1])
                    nc.vector.tensor_add(r_t[:, m, :], ps[:], h_sb[:, m, :])
                    nc.gpsimd.tensor_mul(rsq_t[:, m, :], r_t[:, m, :], r_t[:, m, :])
                h_sb = emit_ln(ep, r_t, rsq_t, l, 0)
                if FP8_FFN1:
                    h28 = ep.tile([P, KD, S], f8, tag="h28", bufs=1)
                    for m in range(KD):
                        nc.gpsimd.tensor_copy(h28[:, m, :], h_sb[:, m, :])
                r_t = ep.tile([P, KD, S], bf16, tag="resid")
                rsq_t = ep.tile([P, KD, S], bf16, tag="rsq")
                for sc in range(SC):
                    sl = slice(sc * 512, (sc + 1) * 512)
                    ff = ep.tile([P, NF, 512], f8 if FP8_FFN2 else bf16,
                                 tag="ffT", bufs=1)
                    for m in range(0, NF, 2):
                        ps = pwide()
                        for j in range(2):
                            half = slice(j * 512, (j + 1) * 512)
                            if FP8_FFN1:
                                for k in range(0, KD, 2):
                                    nc.tensor.matmul(
                                        ps[:, half],
                                        w1_w[:, k:k + 2, (m + j) * P:(m + j + 1) * P],
                                        h28[:, k:k + 2, sl], perf_mode=DR,
                                        start=(k == 0), stop=(k == KD - 2))
                            else:
                                for k in range(KD):
                                    nc.tensor.matmul(
                                        ps[:, half],
                                        w1_w[:, k, (m + j) * P:(m + j + 1) * P],
                                        h_sb[:, k, sl],
                                        start=(k == 0), stop=(k == KD - 1))
                        psv = ps[:].rearrange("p (c s) -> p c s", c=2)
                        if flags["b1"]:
                            nc.vector.tensor_scalar(ff[:, m:m + 2, :], psv,
                                                    b1_c[l][:, m:m + 1], 0.0,
                                                    OP.add, OP.max)
                        elif (m // 2) % 2 == 0:
                            nc.scalar.activation(ff[:, m:m + 2, :], psv, AF.Relu)
                        else:
                            nc.vector.tensor_scalar(ff[:, m:m + 2, :], psv,
                                                    0.0, None, OP.max)
                    for m in range(0, KD, 2):
                        ps = pwide()
                        for j in range(2):
                            half = slice(j * 512, (j + 1) * 512)
                            if FP8_FFN2:
                                for k in range(0, NF, 2):
                                    nc.tensor.matmul(
                                        ps[:, half],
                                        w2_w[:, k:k + 2, (m + j) * P:(m + j + 1) * P],
                                        ff[:, k:k + 2, :], perf_mode=DR,
                                        start=(k == 0), stop=(k == NF - 2))
                            else:
                                for k in range(NF):
                                    nc.tensor.matmul(
                                        ps[:, half],
                                        w2_w[:, k, (m + j) * P:(m + j + 1) * P],
                                        ff[:, k, :],
                                        start=(k == 0), stop=(k == NF - 1))
                        psv = ps[:].rearrange("p (c s) -> p c s", c=2)
                        if flags["b2"]:
                            nc.vector.tensor_scalar_add(psv, psv, b2_c[l][:, m:m + 1])
                        nc.vector.tensor_add(r_t[:, m:m + 2, sl], psv,
                                             h_sb[:, m:m + 2, sl])
                        nc.gpsimd.tensor_mul(rsq_t[:, m:m + 2, sl],
                                             r_t[:, m:m + 2, sl],
                                             r_t[:, m:m + 2, sl])
                return emit_ln(ep, r_t, rsq_t, l, 2)

            def router_phase(rp, h_sb):
                rtq_w = wload(rp, rtq_d, D, D, bf16, "rtq", nsplit=2)
                wout_w = wload(rp, wout_d, D, IN_DIM, bf16, "wout", nsplit=2)
                qrT = rp.tile([P, KD, S], bf16, tag="qrT")
                for m in range(KD):
                    ps = pwide()
                    for k in range(KD):
                        for sc in range(SC):
                            sl = slice(sc * 512, (sc + 1) * 512)
                            nc.tensor.matmul(ps[:, sl], rtq_w[:, k, m * P:(m + 1) * P],
                                             h_sb[:, k, sl],
                                             start=(k == 0), stop=(k == KD - 1))
                    nc.vector.tensor_copy(qrT[:, m, :], ps[:])
                pk = rp.tile([P, 16], f32, tag="pk")
                for sch in range(SCH):
                    ps = pmm()
                    for k in range(KD):
                        nc.tensor.matmul(ps[:, :M], qrT[:, k, sch * P:(sch + 1) * P],
                                         phiT_b[:, k, :],
                                         start=(k == 0), stop=(k == KD - 1))
                    lg = rp.tile([P, M], f32, tag="lgtok", bufs=2)
                    nc.vector.tensor_copy(lg[:], ps[:, :M])
                    mx = rp.tile([P, 8], f32, tag="mx8", bufs=2)
                    nc.vector.max(mx[:], lg[:])
                    e4 = rp.tile([P, 4], f32, tag="e4", bufs=2)
                    nc.vector.tensor_scalar(e4[:], mx[:, 0:4], mx[:, 0:1], None,
                                            OP.subtract)
                    nc.scalar.activation(e4[:], e4[:], AF.Exp)
                    s4 = rp.tile([P, 1], f32, tag="s4", bufs=2)
                    nc.vector.reduce_sum(s4[:], e4[:], axis=mybir.AxisListType.X)
                    nc.scalar.activation(s4[:], s4[:], AF.Ln)
                    nc.vector.tensor_add(s4[:], s4[:], mx[:, 0:1])
                    nc.vector.tensor_scalar_mul(pk[:, sch:sch + 1], s4[:], -1.0)
                    mid = rp.tile([P, 1], f32, tag="mid", bufs=2)
                    nc.vector.tensor_add(mid[:], mx[:, 3:4], mx[:, 4:5])
                    nc.vector.tensor_scalar_mul(pk[:, 8 + sch:9 + sch],
                                                mid[:], 0.5)
                tp_ps = psmall()
                nc.tensor.transpose(tp_ps[:16, :P], pk[:], ident[:])
                t16 = rp.tile([16, P], f32, tag="t16sb")
                nc.vector.tensor_copy(t16[:], tp_ps[:16, :P])
                brow = rp.tile([1, S], f32, tag="brow")
                mrow = rp.tile([1, S], f32, tag="mrow")
                nc.sync.dma_start(brow[:].rearrange("o (c q) -> o c q", c=8),
                                  t16[0:8, :])
                nc.sync.dma_start(mrow[:].rearrange("o (c q) -> o c q", c=8),
                                  t16[8:16, :])
                bias_b = rp.tile([P, S], bf16, tag="biasb")
                mid_b = rp.tile([P, S], bf16, tag="midb")
                for sc in range(SC):
                    sl = slice(sc * 512, (sc + 1) * 512)
                    ps = pmm()
                    nc.tensor.matmul(ps[:], ones_r[:, :P], brow[:, sl],
                                     start=True, stop=True)
                    nc.vector.tensor_copy(bias_b[:, sl], ps[:])
                    ps2 = pmm()
                    nc.tensor.matmul(ps2[:], ones_r[:, :P], mrow[:, sl],
                                     start=True, stop=True)
                    nc.vector.tensor_copy(mid_b[:, sl], ps2[:])
                pt = rp.tile([P, 2, S], bf16, tag="PT")
                for mch in range(2):
                    for sc in range(SC):
                        sl = slice(sc * 512, (sc + 1) * 512)
                        ps = pmm()
                        for k in range(KD):
                            nc.tensor.matmul(ps[:], phiT_b[:, k, mch * P:(mch + 1) * P],
                                             qrT[:, k, sl],
                                             start=(k == 0), stop=(k == KD - 1))
                        t1 = rp.tile([P, 512], f32, tag="ptt1", bufs=2)
                        nc.vector.tensor_add(t1[:], ps[:], bias_b[:, sl])
                        eb = rp.tile([P, 512], bf16, tag="pte", bufs=2)
                        nc.scalar.activation(eb[:], t1[:], AF.Exp)
                        gb = rp.tile([P, 512], bf16, tag="ptg", bufs=2)
                        nc.vector.tensor_tensor(gb[:], ps[:], mid_b[:, sl],
                                                op=OP.is_ge)
                        nc.gpsimd.tensor_mul(pt[:, mch, sl], eb[:], gb[:])
                routed = rp.tile([P, KD, S], bf16, tag="routedT")
                for m in range(KD):
                    ps = pwide()
                    for k in range(2):
                        for sc in range(SC):
                            sl = slice(sc * 512, (sc + 1) * 512)
                            nc.tensor.matmul(ps[:, sl], z_sb[:, k, m * P:(m + 1) * P],
                                             pt[:, k, sl],
                                             start=(k == 0), stop=(k == 1))
                    nc.vector.tensor_add(routed[:, m, :], ps[:], h_sb[:, m, :])
                out_sb = rp.tile([P, 2, S], f32, tag="outT")
                for m in range(2):
                    ps = pwide()
                    for k in range(KD):
                        for sc in range(SC):
                            sl = slice(sc * 512, (sc + 1) * 512)
                            nc.tensor.matmul(ps[:, sl], wout_w[:, k, m * P:(m + 1) * P],
                                             routed[:, k, sl],
                                             start=(k == 0), stop=(k == KD - 1))
                    if flags["bout"]:
                        nc.vector.tensor_scalar_add(out_sb[:, m, :], ps[:],
                                                    bout_c[:, m:m + 1])
                    else:
                        nc.vector.tensor_copy(out_sb[:, m, :], ps[:])
                if reps == 1:
                    nc.sync.dma_start(out_d.rearrange("(o p) s -> p o s", p=P),
                                      out_sb[:])
                else:
                    # timing builds: accumulate so repeated bodies stay live
                    nc.gpsimd.dma_start(out_d.rearrange("(o p) s -> p o s", p=P),
                                        out_sb[:], accum_op=OP.add)

            def body():
                with tc.tile_pool(name="inproj", bufs=1) as ip:
                    h_sb = inproj_phase(ip)
                # bank phase emitted between layers 0 and 1 so its PE work
                # fills the L0->L1 LayerNorm boundary bubble
                with (tc.tile_pool(name="enc", bufs=1) as ep,
                      tc.tile_pool(name="bank", bufs=1) as bp):
                    h_sb = encoder_layer(ep, 0, h_sb)
                    bank_phase(bp)
                    for l in range(1, L):
                        h_sb = encoder_layer(ep, l, h_sb)
                with tc.tile_pool(name="router", bufs=1) as rp:
                    router_phase(rp, h_sb)

            for _ in range(reps):
                body()

    # Pin every activation to the one table set containing Exp+Ln+Relu so
    # the table-load pass emits a single load instead of thrashing between
    # exp_and_others and natural_log (~2.7us per switch).
    import concourse.bacc as bacc_mod
    import concourse.hw_specs as hw_specs_mod
    orig = bacc_mod.get_activation_tables
    keepset = "natural_log_exp_and_others"

    def pinned(arch):
        return {k: (v if k == keepset else set())
                for k, v in hw_specs_mod.get_activation_tables(arch).items()}

    bacc_mod.get_activation_tables = pinned
    try:
        nc.compile()
    finally:
        bacc_mod.get_activation_tables = orig
    return nc


def _flags_from(inputs):
    nz = lambda a: bool(np.any(np.asarray(a)))
    return {
        "bqkv": nz(inputs["enc_bqkv"]),
        "bo": nz(inputs["enc_bo"]),
        "b1": nz(inputs["ff_b1"]),
        "b2": nz(inputs["ff_b2"]),
        "ln": (nz(inputs["ln1_b"]) or nz(inputs["ln2_b"])
               or nz(np.asarray(inputs["ln1_g"]) - 1.0)
               or nz(np.asarray(inputs["ln2_g"]) - 1.0)),
        "bout": nz(inputs["b_out"]),
        "mask": not bool(np.all(np.asarray(inputs["mask"]))),
    }


def _pe_table():
    pos = np.arange(S, dtype=np.float32)[:, None]
    div = np.exp(np.arange(0, D, 2, dtype=np.float32) * (-np.log(10000.0) / D))
    pe = np.zeros((S, D), np.float32)
    pe[:, 0::2] = np.sin(pos * div)
    pe[:, 1::2] = np.cos(pos * div)
    return pe


def make_in_maps(inputs):
    f = np.float32
    b16 = ml_dtypes.bfloat16
    f8t = ml_dtypes.float8_e4m3
    a = {k: np.asarray(v) for k, v in inputs.items()}
    flags = _flags_from(a)
    pe = _pe_table()

    wqkv = a["enc_Wqkv"].astype(f).copy()
    wqkv[:, :, 0:D] *= 1.0 / np.sqrt(DH)          # fold q scale
    sak = a["sa_Wk"].astype(f) * (BETA * ETA / np.sqrt(DH))
    rtq = a["rt_Wq"].astype(f) * (1.0 / np.sqrt(D))

    shared = {
        "win": a["Win"].astype(b16), "wout": a["Wout"].astype(b16),
        "wo4": a["enc_Wo"].astype(b16),
        "saq": a["sa_Wq"].astype(b16), "sak": sak.astype(b16),
        "sav": a["sa_Wv"].astype(b16), "sao": a["sa_Wo"].astype(b16),
        "rtq": rtq.astype(b16),
    }
    if FP8_QK:
        shared["wqk8"] = np.ascontiguousarray(wqkv[:, :, 0:2 * D]).astype(f8t)
        shared["wvb"] = np.ascontiguousarray(wqkv[:, :, 2 * D:3 * D]).astype(b16)
    else:
        shared["wqkv"] = wqkv.astype(b16)
    shared["w18"] = a["ff_W1"].astype(f8t if FP8_FFN1 else b16)
    shared["w28"] = a["ff_W2"].astype(f8t if FP8_FFN2 else b16)
    if flags["bo"]:
        shared["bo4"] = a["enc_bo"].astype(f)
    if flags["b1"]:
        shared["b14"] = a["ff_b1"].astype(f)
    if flags["b2"]:
        shared["b24"] = a["ff_b2"].astype(f)
    if flags["ln"]:
        shared["lnp"] = np.stack(
            [a["ln1_g"], a["ln1_b"], a["ln2_g"], a["ln2_b"]], axis=1).astype(f)
    if flags["bout"]:
        shared["bout"] = a["b_out"].astype(f)

    maps = []
    for b in range(B):
        m = dict(shared)
        m["xT"] = np.ascontiguousarray(a["x_t"][b].T).astype(b16)
        m["pep"] = np.ascontiguousarray(
            pe.T + (a["t_embed"][b] + a["b_in"])[:, None]).astype(b16)
        phi = a["Phi"][b].astype(f)
        m["phiT"] = np.ascontiguousarray(phi.T).astype(b16)
        m["phi2c"] = np.ascontiguousarray(2.0 * C2 * phi.T).astype(b16)
        p2 = np.sum(phi * phi, axis=-1)                       # [M]
        sig = np.mean(a["Sig"][b].astype(f) ** 2, axis=-1)    # [M]
        drow = (GAMMA * np.log(a["Size"][b].astype(f) + 1e-6)
                - 0.5 * sig - C2 * p2)
        m["drow"] = drow.reshape(1, M).astype(f)
        m["p2m"] = (-C2 * p2).reshape(1, M).astype(f)
        maps.append(m)
    return maps, flags


def get_nc(flags, reps=1):
    key = (tuple(sorted(flags.items())), reps)
    if key not in _CACHE:
        _CACHE[key] = _build(flags, reps)
    return _CACHE[key]


def kernel(**inputs):
    from concourse.bass_utils import run_bass_kernel_spmd
    maps, flags = make_in_maps(inputs)
    nc = get_nc(flags, reps=1)
    res = run_bass_kernel_spmd(nc, maps, list(range(B)))
    out = np.stack([np.ascontiguousarray(res.results[b]["outT"].T)
                    for b in range(B)])
    return out.astype(np.float32)
